# revision 16
# baseline (speedup 1.0000x reference)
"""LucasKAN layer kernel for Trainium2 (8 NeuronCores, SPMD data-parallel).

Math: y[b,o] = sum_{i,d} L_d(tanh(x[b,i])) * C[i,o,d],  d = 0..7 (Lucas polys).
Reformulated in the monomial basis: L_d(t) = sum_k A[d,k] t^k with integer A, so
    y[b,o] = bias[o] + sum_{k=1..7} sum_i t^k[b,i] * Cm[k,i,o]
where Cm[k] = sum_d C[:,:,d] A[d,k] (folded on host, exact small-integer combos)
and bias[o] = sum_i Cm[0,i,o]  (the t^0 term needs no matmul).

Degree economization: t^6 and t^7 are nearly inside span{1..t^5} under the
empirical distribution of t = tanh(x) (|t|<1), so both are least-squares
projected onto the lower powers (fit on a subsample of the actual input) and
the projection is folded into Cm[1..5] / bias. This cuts the matmul count from
7 to 5 groups; measured extra error ~1e-3 relative (gate is 2e-2). If the fit
residual is ever large (distribution shift), the exact K=7 program is used.

Per core (1/8 of the batch = 1024 rows):
  - x arrives host-pretransposed per b-chunk as [i%128, i//128, b%128] so the
    contraction dim is on partitions with 4KB DMA lines (no PE transposes)
  - tanh on ACT, powers t^2,t^4 (ACT square), t^3 (DVE), bf16 casts / fused
    bf16-out muls
  - 5 bf16 matmuls per (i-chunk) accumulating into PSUM over i and k
  - warmup: first 3 b-chunks run k-major so matmuls start as soon as the
    first 2MB coefficient tile lands instead of waiting for the full stream
  - bias added on PSUM evacuation (DVE), fp32 result DMA'd out; last chunk
    evacuates per output half to shorten the tail
"""

import sys

for _p in ("/opt/trn_rl_repo",):
    if _p not in sys.path:
        sys.path.insert(0, _p)

import numpy as np
import ml_dtypes

DEGREE = 7
N_CORES = 8
B_FULL, D_IN, D_OUT = 8192, 1024, 1024
B_CORE = B_FULL // N_CORES
P = 128
NB = B_CORE // P  # 8 row-chunks per core
NI = D_IN // P  # 8 contraction chunks
NO = 2  # output split into 2 x 512 (one PSUM bank each)
NF = D_OUT // NO
WARM = 3  # chunks processed k-major while coefficients stream in


def _lucas_monomial_matrix():
    """A[d,k] = coefficient of t^k in L_d(t); L0=2, L1=t, L_d = t*L_{d-1} + L_{d-2}."""
    A = np.zeros((DEGREE + 1, DEGREE + 1), dtype=np.int64)
    A[0, 0] = 2
    A[1, 1] = 1
    for d in range(2, DEGREE + 1):
        A[d, 1:] += A[d - 1, :-1]  # t * L_{d-1}
        A[d] += A[d - 2]
    return A


_CACHE = {}


def _build_program(K):
    """Build the per-core Bass program for K matmul powers (5 or 7)."""
    key = f"nc{K}"
    if key in _CACHE:
        return _CACHE[key]

    from contextlib import ExitStack

    import concourse.bacc as bacc
    import concourse.mybir as mybir
    import concourse.tile as tile

    f32 = mybir.dt.float32
    bf16 = mybir.dt.bfloat16
    AF = mybir.ActivationFunctionType

    # Bacc (not raw Bass): its compile() pass redistributes semaphore waits —
    # TRN2 instructions hold at most one sync wait each.
    nc = bacc.Bacc("TRN2", target_bir_lowering=False, debug=False)
    xt_d = nc.declare_dram_parameter("xt", [NB, P, NI, P], bf16, isOutput=False)
    c2_d = nc.declare_dram_parameter("c2", [K, D_IN, D_OUT], bf16, isOutput=False)
    bias_d = nc.declare_dram_parameter("bias", [P, D_OUT], f32, isOutput=False)
    y_d = nc.declare_dram_parameter("y", [B_CORE, D_OUT], f32, isOutput=True)

    with tile.TileContext(nc) as tc, ExitStack() as ctx:
        const_pool = ctx.enter_context(tc.tile_pool(name="const", bufs=1))
        c2_pool = ctx.enter_context(tc.tile_pool(name="c2p", bufs=1))
        xp = ctx.enter_context(tc.tile_pool(name="xp", bufs=NB))
        ttp = ctx.enter_context(tc.tile_pool(name="ttp", bufs=3))
        fpw = ctx.enter_context(tc.tile_pool(name="fpw", bufs=1))
        pbw = ctx.enter_context(tc.tile_pool(name="pbw", bufs=WARM + 1))
        outp = ctx.enter_context(tc.tile_pool(name="outp", bufs=2))
        ps_acc = ctx.enter_context(tc.tile_pool(name="ps_acc", bufs=3, space="PSUM"))

        xt_sb = [None] * NB
        c2_sb = [None] * K

        def dma_x(b, eng=None):
            t = xp.tile([P, NI, P], bf16, name=f"xt_{b}", tag="xt")
            (eng or nc.sync).dma_start(out=t[:], in_=xt_d[b])
            xt_sb[b] = t

        NIH = NI // 2

        # DMA priority order: x0 + the first coeff half unblock the first
        # matmul group ASAP (each dma_start costs ~0.7us of sync-engine ring
        # dispatch and active rings share bandwidth round-robin, so the
        # critical pieces go first); the x chunks needed during warmup ride
        # between coeff tiles; bias is only needed at the first evacuation.

        def dma_c2_half(k, h):
            if c2_sb[k] is None:
                c2_sb[k] = c2_pool.tile(
                    [P, NI, D_OUT], bf16, name=f"c2k{k}", tag=f"c2k{k}"
                )
            src = c2_d[k].rearrange("(a p) o -> p a o", p=P)
            nc.sync.dma_start(
                out=c2_sb[k][:, h * NIH : (h + 1) * NIH, :],
                in_=src[:, h * NIH : (h + 1) * NIH, :],
            )

        # Ring dispatch parallelized across the three DMA-capable queues:
        # scalar kicks off x0/x1 while sync kicks the coefficient halves
        # (c2_0h0 first — it gates the first matmul group), and the idle
        # gpsimd queue dispatches the later x chunks and bias.
        dma_x(0, nc.scalar)
        dma_x(1, nc.scalar)
        for k in range(K):
            dma_c2_half(k, 0)
            dma_c2_half(k, 1)
        for b in range(2, NB):
            dma_x(b, nc.gpsimd)
        bias_sb = const_pool.tile([P, D_OUT], f32)
        nc.gpsimd.dma_start(out=bias_sb[:], in_=bias_d[:, :])

        def powers(b):
            """tanh + monomial powers for chunk b, bf16 pw[k] tiles k=1..K.

            Emission order puts each pk as early as its first matmul needs
            it (p1 right after tanh for the warmup k=1 groups)."""
            xt = xt_sb[b]
            t1 = ttp.tile([P, NI, P], f32, name=f"t1_{b}", tag="t1")
            pw = []

            def cast_act(src, k):
                pk = pbw.tile([P, NI, P], bf16, name=f"p{k}_{b}", tag=f"p{k}")
                nc.scalar.copy(pk[:], src[:])
                pw.append(pk)

            def mul_dve(a, c, k):
                pk = pbw.tile([P, NI, P], bf16, name=f"p{k}_{b}", tag=f"p{k}")
                nc.vector.tensor_mul(pk[:], a[:], c[:])
                pw.append(pk)

            # tanh + p1 in i-halves: the warmup's first matmul group only
            # needs the first half, which trims the startup critical path.
            p1 = pbw.tile([P, NI, P], bf16, name=f"p1_{b}", tag="p1")
            for h in range(2):
                sl = slice(h * NIH, (h + 1) * NIH)
                nc.scalar.activation(t1[:, sl], xt[:, sl], AF.Tanh)
                nc.scalar.copy(p1[:, sl], t1[:, sl])
            pw.append(p1)
            t2 = fpw.tile([P, NI, P], f32, name=f"t2_{b}", tag="t2")
            nc.scalar.square(t2[:], t1[:])
            cast_act(t2, 2)
            t3 = fpw.tile([P, NI, P], f32, name=f"t3_{b}", tag="t3")
            nc.vector.tensor_mul(t3[:], t1[:], t2[:])
            p3 = pbw.tile([P, NI, P], bf16, name=f"p3_{b}", tag="p3")
            nc.vector.tensor_copy(p3[:], t3[:])
            pw.append(p3)
            t4 = fpw.tile([P, NI, P], f32, name=f"t4_{b}", tag="t4")
            nc.scalar.square(t4[:], t2[:])
            cast_act(t4, 4)
            mul_dve(t1, t4, 5)
            if K > 5:
                mul_dve(t2, t4, 6)
                mul_dve(t3, t4, 7)
            return pw

        def new_accs(b):
            return [
                ps_acc.tile([P, NF], f32, name=f"acc_{b}_{o}", tag=f"acc{o}")
                for o in range(NO)
            ]

        def evac(b, accs, o, split=1):
            # split=2 halves the add+store so the first store overlaps the
            # second add — only worth it on the final evacuation's tail.
            out_sb = outp.tile([P, NF], f32, name=f"out_{b}_{o}", tag=f"out{o}")
            w = NF // split
            for s in range(split):
                sl = slice(s * w, (s + 1) * w)
                nc.vector.tensor_add(
                    out_sb[:, sl],
                    accs[o][:, sl],
                    bias_sb[:, o * NF + s * w : o * NF + (s + 1) * w],
                )
                nc.sync.dma_start(
                    out=y_d[b * P : (b + 1) * P, o * NF + s * w : o * NF + (s + 1) * w],
                    in_=out_sb[:, sl],
                )

        # Warmup: first WARM chunks k-major so each 2MB coeff tile is consumed
        # by 3 chunks' worth of matmuls (~10.8us) while the next tile streams
        # in (~5.6us) — the PE never waits past the first tile.
        pw_w = [powers(b) for b in range(WARM)]
        accs_w = [new_accs(b) for b in range(WARM)]
        for k in range(K):
            for h in range(2):
                for b in range(WARM):
                    for i in range(h * NIH, (h + 1) * NIH):
                        for o in range(NO):
                            nc.tensor.matmul(
                                accs_w[b][o][:],
                                pw_w[b][k][:, i, :],
                                c2_sb[k][:, i, o * NF : (o + 1) * NF],
                                start=(k == 0 and i == 0),
                                stop=(k == K - 1 and i == NI - 1),
                            )
                    if k == K - 1 and h == 1:
                        for o in range(NO):
                            evac(b, accs_w[b], o)

        # Steady state: one chunk at a time, all coefficients resident.
        for b in range(WARM, NB):
            pw = powers(b)
            accs = new_accs(b)
            if b < NB - 1:
                for k in range(K):
                    for i in range(NI):
                        for o in range(NO):
                            nc.tensor.matmul(
                                accs[o][:],
                                pw[k][:, i, :],
                                c2_sb[k][:, i, o * NF : (o + 1) * NF],
                                start=(k == 0 and i == 0),
                                stop=(k == K - 1 and i == NI - 1),
                            )
                for o in range(NO):
                    evac(b, accs, o)
            else:
                # Last chunk: finish output half o=0 first so its evacuation
                # and store overlap the o=1 matmuls instead of trailing them.
                for o in range(NO):
                    for k in range(K):
                        for i in range(NI):
                            nc.tensor.matmul(
                                accs[o][:],
                                pw[k][:, i, :],
                                c2_sb[k][:, i, o * NF : (o + 1) * NF],
                                start=(k == 0 and i == 0),
                                stop=(k == K - 1 and i == NI - 1),
                            )
                    evac(b, accs, o, split=2 if o == NO - 1 else 1)

    if not nc.is_finalized():
        nc.finalize()
    _CACHE[key] = nc
    return nc


def _economize(Cm, x):
    """LS-project t^6,t^7 onto {1..t^5} under the empirical tanh(x) dist.

    Returns (Cm2[k=0..5], ok). ok=False if the residuals are too large for
    the folded 6-power form to stay well inside the accuracy gate."""
    rng = np.random.default_rng(0)
    flat = x.reshape(-1)
    n = min(200_000, flat.size)
    idx = rng.choice(flat.size, n, replace=False) if flat.size > n else slice(None)
    tf = np.tanh(flat[idx].astype(np.float64))
    V = np.stack([tf**k for k in range(6)], axis=1)
    Cm2 = Cm[:6].astype(np.float64).copy()
    # scale of y per unit coeff-variance: contributions add in quadrature
    tot_var = sum(
        float(np.mean((tf ** k) ** 2)) * float(np.var(Cm[k])) for k in range(1, 8)
    )
    err_var = 0.0
    for kk in (6, 7):
        yk = tf**kk
        coef, *_ = np.linalg.lstsq(V, yk, rcond=None)
        resid = yk - V @ coef
        err_var += float(np.mean(resid**2)) * float(np.var(Cm[kk]))
        for k in range(6):
            Cm2[k] += coef[k] * Cm[kk].astype(np.float64)
    # predicted rms relative error from economization alone
    pred_rel = np.sqrt(err_var / max(tot_var, 1e-30))
    return Cm2.astype(np.float32), bool(pred_rel < 5e-3)


def _prepare(x, lucas_coeffs):
    """Host prep: fold Lucas->monomial coeffs, economize, shard inputs."""
    A = _lucas_monomial_matrix().astype(np.float32)
    Cm = np.einsum("iod,dk->kio", lucas_coeffs.astype(np.float32), A)
    x = np.ascontiguousarray(x, dtype=np.float32).reshape(B_FULL, D_IN)

    Cm2, ok = _economize(Cm, x)
    if ok:
        K = 5
        c_use, c0 = Cm2[1:6], Cm2[0]
    else:
        K = DEGREE
        c_use, c0 = Cm[1:], Cm[0]

    bias = c0.sum(axis=0, dtype=np.float32)  # [D_OUT]
    bias_rep = np.ascontiguousarray(np.broadcast_to(bias, (P, D_OUT)), dtype=np.float32)
    c2 = np.ascontiguousarray(c_use).astype(ml_dtypes.bfloat16)

    # Per-core x slab, pre-transposed: [b_chunk, i%128, i//128, b%128] so each
    # chunk DMA is 128 partitions x 2KB contiguous lines. bf16: tanh-input
    # quantization adds ~the same noise as the bf16 power casts (~1e-3 rel)
    # and halves the startup-critical x bytes.
    xb = x.astype(ml_dtypes.bfloat16)
    in_maps = []
    for c in range(N_CORES):
        slab = xb[c * B_CORE : (c + 1) * B_CORE]  # [B_CORE, D_IN]
        xt = np.ascontiguousarray(
            slab.T.reshape(NI, P, NB, P).transpose(2, 1, 0, 3)
        )  # [NB, P(i%128), NI, P(b%128)]
        in_maps.append({"xt": xt, "c2": c2, "bias": bias_rep})
    return K, in_maps


def kernel(x: np.ndarray, lucas_coeffs: np.ndarray) -> np.ndarray:
    from concourse.bass_utils import run_bass_kernel_spmd

    K, in_maps = _prepare(x, lucas_coeffs)
    nc = _build_program(K)
    res = run_bass_kernel_spmd(nc, in_maps, list(range(N_CORES)))
    return np.concatenate([r["y"] for r in res.results], axis=0)
